# revision 14
# baseline (speedup 1.0000x reference)
"""AdaFace margin loss on 8 trn2 NeuronCores (class-dim sharded, partial-FC style).

Key identity: for non-label entries the reference computes
cos(arccos(c)) * S == c * S, so the bulk of the [512, 85742] output is a
pure scale -- memory-bound streaming. Only the <=512 label positions need
the margin path, computed as cos(arccos(c) + g) = c*cos(g) - sqrt(1-c^2)*sin(g)
and applied via an indirect-DMA gather/scatter. Rows whose label falls in
another core's shard point their index at a dummy row ([B] extra) so every
lane stays in bounds and the margin lands only where it belongs.
"""

import math

import numpy as np

B = 512          # batch
C = 85742        # classes (global)
NCORES = 8
CS = 10720       # per-core shard width; 8*CS = 85760 >= C (padded)
P = 128          # SBUF partitions
NB = B // P      # 4 batches of 128 rows for gather/scatter lanes
W = 5360         # bulk tile free width; (B*CS) / (P*W) tiles per core
BULK_BUFS = 4

M_CONST = 0.4
H_CONST = 0.333
S_CONST = 64.0
T_ALPHA = 0.01
EPS = 0.001

_NC_CACHE = {}


def build_nc(w=W, bulk_bufs=BULK_BUFS):
    import concourse.bass as bass
    import concourse.mybir as mybir
    from concourse.bacc import Bacc
    from concourse.tile import TileContext

    f32 = mybir.dt.float32
    i32 = mybir.dt.int32
    Alu = mybir.AluOpType
    Act = mybir.ActivationFunctionType
    X = mybir.AxisListType.X

    ntiles = (B * CS) // (P * w)
    assert ntiles * P * w == B * CS

    nc = Bacc("TRN2", target_bir_lowering=False)
    # inputs/output carry one dummy row: out-of-shard labels gather from and
    # scatter to it, so no index is ever out of bounds.
    cos_d = nc.declare_dram_parameter("cosine", [B + 1, CS], f32, isOutput=False)
    nrm_d = nc.declare_dram_parameter("norms", [1, B], f32, isOutput=False)
    bm_d = nc.declare_dram_parameter("batch_mean", [1, 1], f32, isOutput=False)
    bs_d = nc.declare_dram_parameter("batch_std", [1, 1], f32, isOutput=False)
    fidx_d = nc.declare_dram_parameter("fidx", [P, NB], i32, isOutput=False)
    out_d = nc.declare_dram_parameter("out", [B + 1, CS], f32, isOutput=True)

    cos_flat = cos_d[:, :].rearrange("b (c u) -> (b c) u", u=1)
    out_flat = out_d[:, :].rearrange("b (c u) -> (b c) u", u=1)
    cos_tiled = cos_d[:B, :].rearrange("b c -> (b c)").rearrange(
        "(t p w) -> t p w", p=P, w=w
    )
    out_tiled = out_d[:B, :].rearrange("b c -> (b c)").rearrange(
        "(t p w) -> t p w", p=P, w=w
    )

    with TileContext(nc) as tc:
        with (
            tc.tile_pool(name="small", bufs=1) as sp,
            tc.tile_pool(name="bulk", bufs=bulk_bufs) as bp,
        ):
            # ---- tiny inputs ------------------------------------------------
            n_t = sp.tile([1, B], f32)
            nc.sync.dma_start(out=n_t[:], in_=nrm_d[:, :])
            bm_t = sp.tile([1, 1], f32)
            nc.sync.dma_start(out=bm_t[:], in_=bm_d[:, :])
            bs_t = sp.tile([1, 1], f32)
            nc.sync.dma_start(out=bs_t[:], in_=bs_d[:, :])
            fx_t = sp.tile([P, NB], i32)
            nc.sync.dma_start(out=fx_t[:], in_=fidx_d[:, :])

            # ---- norm statistics (safe_norms mean / unbiased std) -----------
            nc.vector.tensor_scalar(
                out=n_t[:], in0=n_t[:], scalar1=0.001, scalar2=100.0,
                op0=Alu.max, op1=Alu.min,
            )
            sum_t = sp.tile([1, 1], f32)
            nc.vector.reduce_sum(out=sum_t[:], in_=n_t[:], axis=X)
            mean_t = sp.tile([1, 1], f32)
            nc.vector.tensor_scalar_mul(mean_t[:], sum_t[:], 1.0 / B)
            cen_t = sp.tile([1, B], f32)
            nc.vector.tensor_tensor(
                out=cen_t[:], in0=n_t[:], in1=mean_t[:].to_broadcast([1, B]),
                op=Alu.subtract,
            )
            sq_t = sp.tile([1, B], f32)
            nc.scalar.activation(sq_t[:], cen_t[:], Act.Square)
            vs_t = sp.tile([1, 1], f32)
            nc.vector.reduce_sum(out=vs_t[:], in_=sq_t[:], axis=X)
            std_t = sp.tile([1, 1], f32)
            nc.scalar.activation(std_t[:], vs_t[:], Act.Sqrt, scale=1.0 / (B - 1))

            # ---- EMA buffers + margin scaler --------------------------------
            nm_t = sp.tile([1, 1], f32)
            nc.vector.tensor_scalar_mul(nm_t[:], mean_t[:], T_ALPHA)
            tb_t = sp.tile([1, 1], f32)
            nc.vector.tensor_scalar_mul(tb_t[:], bm_t[:], 1.0 - T_ALPHA)
            nc.vector.tensor_add(nm_t[:], nm_t[:], tb_t[:])
            dn_t = sp.tile([1, 1], f32)
            nc.vector.tensor_scalar(
                out=dn_t[:], in0=std_t[:], scalar1=T_ALPHA, scalar2=EPS,
                op0=Alu.mult, op1=Alu.add,
            )
            ts_t = sp.tile([1, 1], f32)
            nc.vector.tensor_scalar_mul(ts_t[:], bs_t[:], 1.0 - T_ALPHA)
            nc.vector.tensor_add(dn_t[:], dn_t[:], ts_t[:])
            inv_t = sp.tile([1, 1], f32)
            nc.vector.reciprocal(inv_t[:], dn_t[:])

            ms_t = sp.tile([1, B], f32)
            nc.vector.tensor_tensor(
                out=ms_t[:], in0=n_t[:], in1=nm_t[:].to_broadcast([1, B]),
                op=Alu.subtract,
            )
            nc.vector.tensor_tensor(
                out=ms_t[:], in0=ms_t[:], in1=inv_t[:].to_broadcast([1, B]),
                op=Alu.mult,
            )
            nc.vector.tensor_scalar(
                out=ms_t[:], in0=ms_t[:], scalar1=H_CONST, scalar2=None,
                op0=Alu.mult,
            )
            nc.vector.tensor_scalar(
                out=ms_t[:], in0=ms_t[:], scalar1=-1.0, scalar2=1.0,
                op0=Alu.max, op1=Alu.min,
            )
            ga_t = sp.tile([1, B], f32)
            nc.vector.tensor_scalar_mul(ga_t[:], ms_t[:], -M_CONST)
            cosg_t = sp.tile([1, B], f32)
            nc.vector.tensor_scalar_add(cosg_t[:], ga_t[:], math.pi / 2.0)
            nc.scalar.activation(cosg_t[:], cosg_t[:], Act.Sin)
            sing_t = sp.tile([1, B], f32)
            nc.scalar.activation(sing_t[:], ga_t[:], Act.Sin)
            gadd_t = sp.tile([1, B], f32)
            nc.vector.tensor_scalar(
                out=gadd_t[:], in0=ms_t[:], scalar1=M_CONST, scalar2=M_CONST,
                op0=Alu.mult, op1=Alu.add,
            )

            # ---- redistribute per-row vectors [1,512] -> [128,4] ------------
            # Row b = i*128+p lands at (p, i).  SBUF partition redistribution
            # is not expressible as a direct SBUF->SBUF access pattern, so
            # bounce through a DRAM scratch (DRAM APs are plain linear).
            scratch_d = nc.dram_tensor("redist_scratch", [3, B], f32)
            nc.sync.dma_start(out=scratch_d[0:1, :], in_=cosg_t[:])
            nc.sync.dma_start(out=scratch_d[1:2, :], in_=sing_t[:])
            nc.sync.dma_start(out=scratch_d[2:3, :], in_=gadd_t[:])
            vpi_t = sp.tile([P, 3 * NB], f32)
            nc.sync.dma_start(
                out=vpi_t[:].rearrange("p (v i) -> p v i", v=3),
                in_=scratch_d[:, :].rearrange("v (i p) -> p v i", p=P),
            )
            cosg_pi = vpi_t[:, 0:NB]
            sing_pi = vpi_t[:, NB : 2 * NB]
            gadd_pi = vpi_t[:, 2 * NB : 3 * NB]

            # ---- gather label cosines (one [128,1] gather per row batch) ----
            clab_t = sp.tile([P, NB], f32)
            for i in range(NB):
                nc.gpsimd.indirect_dma_start(
                    out=clab_t[:, i : i + 1],
                    out_offset=None,
                    in_=cos_flat,
                    in_offset=bass.IndirectOffsetOnAxis(
                        ap=fx_t[:, i : i + 1], axis=0
                    ),
                )

            # ---- corrected logits at label positions ------------------------
            c2_t = sp.tile([P, NB], f32)
            nc.scalar.activation(c2_t[:], clab_t[:], Act.Square)
            sn_t = sp.tile([P, NB], f32)
            nc.scalar.activation(sn_t[:], c2_t[:], Act.Sqrt, bias=1.0, scale=-1.0)
            corr_t = sp.tile([P, NB], f32)
            nc.vector.tensor_mul(corr_t[:], clab_t[:], cosg_pi)
            nc.vector.tensor_mul(sn_t[:], sn_t[:], sing_pi)
            nc.vector.tensor_sub(corr_t[:], corr_t[:], sn_t[:])
            nc.vector.tensor_sub(corr_t[:], corr_t[:], gadd_pi)
            nc.vector.tensor_scalar_mul(corr_t[:], corr_t[:], S_CONST)

            # ---- bulk: out = cosine * S ------------------------------------
            for t in range(ntiles):
                tile = bp.tile([P, w], f32)
                nc.sync.dma_start(out=tile[:], in_=cos_tiled[t, :, :])
                nc.vector.tensor_scalar_mul(tile[:], tile[:], S_CONST)
                nc.sync.dma_start(out=out_tiled[t, :, :], in_=tile[:])

            # ---- scatter corrections over the bulk output -------------------
            # Tile tracks the DRAM WAW vs the bulk out-DMAs; Bacc's
            # generate_event_semaphores legalizes the resulting multi-wait.
            for i in range(NB):
                nc.gpsimd.indirect_dma_start(
                    out=out_flat,
                    out_offset=bass.IndirectOffsetOnAxis(
                        ap=fx_t[:, i : i + 1], axis=0
                    ),
                    in_=corr_t[:, i : i + 1],
                    in_offset=None,
                )

    nc.finalize()  # Bacc.compile(): splits >1-wait instructions into EventSemaphores
    return nc


def get_nc():
    if "nc" not in _NC_CACHE:
        _NC_CACHE["nc"] = build_nc()
    return _NC_CACHE["nc"]


def shard_inputs(cosine, norms, batch_mean, batch_std, label):
    cosine = np.ascontiguousarray(np.asarray(cosine), dtype=np.float32)
    norms = np.ascontiguousarray(
        np.asarray(norms, dtype=np.float32).reshape(1, B)
    )
    bm = np.asarray(batch_mean, dtype=np.float32).reshape(1, 1)
    bs = np.asarray(batch_std, dtype=np.float32).reshape(1, 1)
    lab = np.asarray(label).astype(np.int64).reshape(B)
    b_idx = np.arange(B, dtype=np.int64)

    in_maps = []
    for k in range(NCORES):
        lo = k * CS
        hi = min(lo + CS, C)
        shard = np.zeros((B + 1, CS), dtype=np.float32)
        shard[:B, : hi - lo] = cosine[:, lo:hi]
        loc = lab - lo
        ok = (lab != -1) & (loc >= 0) & (loc < CS)
        # invalid rows target the dummy row (index B*CS + b): always in
        # bounds, never part of the real output
        fidx = np.where(ok, b_idx * CS + loc, B * CS + b_idx).astype(np.int32)
        fidx_pi = np.ascontiguousarray(fidx.reshape(NB, P).T)  # row i*128+p -> (p,i)
        in_maps.append(
            {
                "cosine": shard,
                "norms": norms,
                "batch_mean": bm,
                "batch_std": bs,
                "fidx": fidx_pi,
            }
        )
    return in_maps


def unshard_output(outs):
    full = np.empty((B, C), dtype=np.float32)
    for k in range(NCORES):
        lo = k * CS
        hi = min(lo + CS, C)
        full[:, lo:hi] = outs[k]["out"][:B, : hi - lo]
    return full


def run_on_hw(in_maps, trace=False, **kwargs):
    from concourse.bass_utils import run_bass_kernel_spmd

    nc = get_nc()
    return run_bass_kernel_spmd(
        nc, in_maps, core_ids=list(range(NCORES)), trace=trace, **kwargs
    )


def kernel(cosine, norms, batch_mean, batch_std, label):
    in_maps = shard_inputs(cosine, norms, batch_mean, batch_std, label)
    res = run_on_hw(in_maps)
    return unshard_output(res.results)


# revision 17
# speedup vs baseline: 1.1059x; 1.1059x over previous
"""AdaFace margin loss on 8 trn2 NeuronCores (class-dim sharded, partial-FC style).

Key identity: for non-label entries the reference computes
cos(arccos(c)) * S == c * S, so the bulk of the [512, 85742] output is a
pure scale -- memory-bound streaming. Only the <=512 label positions need
the margin path, computed as cos(arccos(c) + g) = c*cos(g) - sqrt(1-c^2)*sin(g).

Label cosines are fetched with an indirect-DMA gather (out-of-shard labels
read a dummy input row so every index is in bounds); the corrected logits
are merged into the bulk tiles in SBUF via an iota==label_col predicated
copy before each tile is written out -- no scatter, so the output stream
never stalls on a read-modify-write tail.
"""

import math

import numpy as np

B = 512          # batch
C = 85742        # classes (global)
NCORES = 8
CS = 10720       # per-core shard width; 8*CS = 85760 >= C (padded)
P = 128          # SBUF partitions
NB = B // P      # 4 row blocks of 128 rows
W = 5360         # bulk tile free width
NCT = CS // W    # col tiles per row block
BULK_BUFS = 4
MASK_BUFS = 2

M_CONST = 0.4
H_CONST = 0.333
S_CONST = 64.0
T_ALPHA = 0.01
EPS = 0.001

_NC_CACHE = {}


def build_nc(w=W, bulk_bufs=BULK_BUFS):
    import concourse.bass as bass
    import concourse.mybir as mybir
    from concourse.bacc import Bacc
    from concourse.tile import TileContext

    f32 = mybir.dt.float32
    i32 = mybir.dt.int32
    Alu = mybir.AluOpType
    Act = mybir.ActivationFunctionType
    X = mybir.AxisListType.X

    nct = CS // w
    assert nct * w == CS

    nc = Bacc("TRN2", target_bir_lowering=False)
    # cosine carries one dummy row: out-of-shard labels gather from it, so
    # no gather index is ever out of bounds.
    cos_d = nc.declare_dram_parameter("cosine", [B + 1, CS], f32, isOutput=False)
    nrm_d = nc.declare_dram_parameter("norms", [1, B], f32, isOutput=False)
    bm_d = nc.declare_dram_parameter("batch_mean", [1, 1], f32, isOutput=False)
    bs_d = nc.declare_dram_parameter("batch_std", [1, 1], f32, isOutput=False)
    fidx_d = nc.declare_dram_parameter("fidx", [P, NB], i32, isOutput=False)
    # colf[p, rb*nct+ct] = label col of row rb*128+p minus ct*w, or -1 when
    # the label is not in this shard/tile
    colf_d = nc.declare_dram_parameter("colf", [P, NB * nct], f32, isOutput=False)
    out_d = nc.declare_dram_parameter("out", [B, CS], f32, isOutput=True)

    cos_flat = cos_d[:, :].rearrange("b (c u) -> (b c) u", u=1)

    with TileContext(nc) as tc:
        with (
            tc.tile_pool(name="small", bufs=1) as sp,
            tc.tile_pool(name="bulk", bufs=bulk_bufs) as bp,
            tc.tile_pool(name="mask", bufs=MASK_BUFS) as mp,
        ):
            # ---- constants / tiny inputs ------------------------------------
            iota_t = sp.tile([P, w], f32)
            nc.gpsimd.iota(iota_t[:], pattern=[[1, w]], base=0, channel_multiplier=0,
                           allow_small_or_imprecise_dtypes=True)
            n_t = sp.tile([1, B], f32)
            nc.sync.dma_start(out=n_t[:], in_=nrm_d[:, :])
            bm_t = sp.tile([1, 1], f32)
            nc.sync.dma_start(out=bm_t[:], in_=bm_d[:, :])
            bs_t = sp.tile([1, 1], f32)
            nc.sync.dma_start(out=bs_t[:], in_=bs_d[:, :])
            fx_t = sp.tile([P, NB], i32)
            nc.sync.dma_start(out=fx_t[:], in_=fidx_d[:, :])
            cf_t = sp.tile([P, NB * nct], f32)
            nc.sync.dma_start(out=cf_t[:], in_=colf_d[:, :])

            # ---- norm statistics (safe_norms mean / unbiased std) -----------
            nc.vector.tensor_scalar(
                out=n_t[:], in0=n_t[:], scalar1=0.001, scalar2=100.0,
                op0=Alu.max, op1=Alu.min,
            )
            sum_t = sp.tile([1, 1], f32)
            nc.vector.reduce_sum(out=sum_t[:], in_=n_t[:], axis=X)
            mean_t = sp.tile([1, 1], f32)
            nc.vector.tensor_scalar_mul(mean_t[:], sum_t[:], 1.0 / B)
            cen_t = sp.tile([1, B], f32)
            nc.vector.tensor_tensor(
                out=cen_t[:], in0=n_t[:], in1=mean_t[:].to_broadcast([1, B]),
                op=Alu.subtract,
            )
            sq_t = sp.tile([1, B], f32)
            nc.scalar.activation(sq_t[:], cen_t[:], Act.Square)
            vs_t = sp.tile([1, 1], f32)
            nc.vector.reduce_sum(out=vs_t[:], in_=sq_t[:], axis=X)
            std_t = sp.tile([1, 1], f32)
            nc.scalar.activation(std_t[:], vs_t[:], Act.Sqrt, scale=1.0 / (B - 1))

            # ---- EMA buffers + margin scaler --------------------------------
            nm_t = sp.tile([1, 1], f32)
            nc.vector.tensor_scalar_mul(nm_t[:], mean_t[:], T_ALPHA)
            tb_t = sp.tile([1, 1], f32)
            nc.vector.tensor_scalar_mul(tb_t[:], bm_t[:], 1.0 - T_ALPHA)
            nc.vector.tensor_add(nm_t[:], nm_t[:], tb_t[:])
            dn_t = sp.tile([1, 1], f32)
            nc.vector.tensor_scalar(
                out=dn_t[:], in0=std_t[:], scalar1=T_ALPHA, scalar2=EPS,
                op0=Alu.mult, op1=Alu.add,
            )
            ts_t = sp.tile([1, 1], f32)
            nc.vector.tensor_scalar_mul(ts_t[:], bs_t[:], 1.0 - T_ALPHA)
            nc.vector.tensor_add(dn_t[:], dn_t[:], ts_t[:])
            inv_t = sp.tile([1, 1], f32)
            nc.vector.reciprocal(inv_t[:], dn_t[:])

            ms_t = sp.tile([1, B], f32)
            nc.vector.tensor_tensor(
                out=ms_t[:], in0=n_t[:], in1=nm_t[:].to_broadcast([1, B]),
                op=Alu.subtract,
            )
            nc.vector.tensor_tensor(
                out=ms_t[:], in0=ms_t[:], in1=inv_t[:].to_broadcast([1, B]),
                op=Alu.mult,
            )
            nc.vector.tensor_scalar(
                out=ms_t[:], in0=ms_t[:], scalar1=H_CONST, scalar2=None,
                op0=Alu.mult,
            )
            nc.vector.tensor_scalar(
                out=ms_t[:], in0=ms_t[:], scalar1=-1.0, scalar2=1.0,
                op0=Alu.max, op1=Alu.min,
            )
            ga_t = sp.tile([1, B], f32)
            nc.vector.tensor_scalar_mul(ga_t[:], ms_t[:], -M_CONST)
            cosg_t = sp.tile([1, B], f32)
            nc.vector.tensor_scalar_add(cosg_t[:], ga_t[:], math.pi / 2.0)
            nc.scalar.activation(cosg_t[:], cosg_t[:], Act.Sin)
            sing_t = sp.tile([1, B], f32)
            nc.scalar.activation(sing_t[:], ga_t[:], Act.Sin)
            gadd_t = sp.tile([1, B], f32)
            nc.vector.tensor_scalar(
                out=gadd_t[:], in0=ms_t[:], scalar1=M_CONST, scalar2=M_CONST,
                op0=Alu.mult, op1=Alu.add,
            )

            # ---- redistribute per-row vectors [1,512] -> [128,4] ------------
            # Row b = rb*128+p lands at (p, rb).  SBUF partition
            # redistribution is not expressible as a direct SBUF->SBUF access
            # pattern, so bounce through a DRAM scratch (DRAM is linear).
            scratch_d = nc.dram_tensor("redist_scratch", [3, B], f32)
            nc.sync.dma_start(out=scratch_d[0:1, :], in_=cosg_t[:])
            nc.sync.dma_start(out=scratch_d[1:2, :], in_=sing_t[:])
            nc.sync.dma_start(out=scratch_d[2:3, :], in_=gadd_t[:])
            vpi_t = sp.tile([P, 3 * NB], f32)
            nc.sync.dma_start(
                out=vpi_t[:].rearrange("p (v i) -> p v i", v=3),
                in_=scratch_d[:, :].rearrange("v (i p) -> p v i", p=P),
            )
            cosg_pi = vpi_t[:, 0:NB]
            sing_pi = vpi_t[:, NB : 2 * NB]
            gadd_pi = vpi_t[:, 2 * NB : 3 * NB]

            # ---- gather label cosines (one [128,1] gather per row block) ----
            clab_t = sp.tile([P, NB], f32)
            for i in range(NB):
                nc.gpsimd.indirect_dma_start(
                    out=clab_t[:, i : i + 1],
                    out_offset=None,
                    in_=cos_flat,
                    in_offset=bass.IndirectOffsetOnAxis(
                        ap=fx_t[:, i : i + 1], axis=0
                    ),
                )

            # ---- corrected logits at label positions ------------------------
            c2_t = sp.tile([P, NB], f32)
            nc.scalar.activation(c2_t[:], clab_t[:], Act.Square)
            sn_t = sp.tile([P, NB], f32)
            nc.scalar.activation(sn_t[:], c2_t[:], Act.Sqrt, bias=1.0, scale=-1.0)
            corr_t = sp.tile([P, NB], f32)
            nc.vector.tensor_mul(corr_t[:], clab_t[:], cosg_pi)
            nc.vector.tensor_mul(sn_t[:], sn_t[:], sing_pi)
            nc.vector.tensor_sub(corr_t[:], corr_t[:], sn_t[:])
            nc.vector.tensor_sub(corr_t[:], corr_t[:], gadd_pi)
            nc.vector.tensor_scalar_mul(corr_t[:], corr_t[:], S_CONST)

            # ---- bulk: out = cosine * S, with the label column overwritten
            # by the corrected logit before the tile is stored ---------------
            for rb in range(NB):
                rows = slice(rb * P, (rb + 1) * P)
                for ct in range(nct):
                    cols = slice(ct * w, (ct + 1) * w)
                    tidx = rb * nct + ct
                    tile = bp.tile([P, w], f32)
                    nc.sync.dma_start(out=tile[:], in_=cos_d[rows, cols])
                    nc.scalar.mul(tile[:], tile[:], S_CONST)
                    mask_t = mp.tile([P, w], mybir.dt.uint32)
                    nc.vector.tensor_scalar(
                        out=mask_t[:], in0=iota_t[:],
                        scalar1=cf_t[:, tidx : tidx + 1], scalar2=None,
                        op0=Alu.is_equal,
                    )
                    nc.vector.copy_predicated(
                        tile[:], mask_t[:],
                        corr_t[:, rb : rb + 1].to_broadcast([P, w]),
                    )
                    nc.sync.dma_start(out=out_d[rows, cols], in_=tile[:])

    nc.finalize()  # Bacc.compile(): splits >1-wait instructions into EventSemaphores
    return nc


def get_nc():
    if "nc" not in _NC_CACHE:
        _NC_CACHE["nc"] = build_nc()
    return _NC_CACHE["nc"]


def shard_inputs(cosine, norms, batch_mean, batch_std, label):
    cosine = np.ascontiguousarray(np.asarray(cosine), dtype=np.float32)
    norms = np.ascontiguousarray(
        np.asarray(norms, dtype=np.float32).reshape(1, B)
    )
    bm = np.asarray(batch_mean, dtype=np.float32).reshape(1, 1)
    bs = np.asarray(batch_std, dtype=np.float32).reshape(1, 1)
    lab = np.asarray(label).astype(np.int64).reshape(B)
    b_idx = np.arange(B, dtype=np.int64)
    nct = CS // W

    in_maps = []
    for k in range(NCORES):
        lo = k * CS
        hi = min(lo + CS, C)
        shard = np.zeros((B + 1, CS), dtype=np.float32)
        shard[:B, : hi - lo] = cosine[:, lo:hi]
        loc = lab - lo
        ok = (lab != -1) & (loc >= 0) & (loc < CS)
        # gather index; invalid rows read the dummy row (always in bounds)
        fidx = np.where(ok, b_idx * CS + loc, B * CS + b_idx).astype(np.int32)
        fidx_pi = np.ascontiguousarray(fidx.reshape(NB, P).T)
        # per-tile label column (or -1): row rb*128+p, tile tidx=rb*nct+ct
        colf = np.full((P, NB * nct), -1, dtype=np.float32)
        for rb in range(NB):
            rows = np.arange(rb * P, (rb + 1) * P)
            for ct in range(nct):
                cc = loc[rows] - ct * W
                m = ok[rows] & (cc >= 0) & (cc < W)
                colf[:, rb * nct + ct] = np.where(m, cc, -1).astype(np.float32)
        in_maps.append(
            {
                "cosine": shard,
                "norms": norms,
                "batch_mean": bm,
                "batch_std": bs,
                "fidx": fidx_pi,
                "colf": np.ascontiguousarray(colf),
            }
        )
    return in_maps


def unshard_output(outs):
    full = np.empty((B, C), dtype=np.float32)
    for k in range(NCORES):
        lo = k * CS
        hi = min(lo + CS, C)
        full[:, lo:hi] = outs[k]["out"][:B, : hi - lo]
    return full


def run_on_hw(in_maps, trace=False, **kwargs):
    from concourse.bass_utils import run_bass_kernel_spmd

    nc = get_nc()
    return run_bass_kernel_spmd(
        nc, in_maps, core_ids=list(range(NCORES)), trace=trace, **kwargs
    )


def kernel(cosine, norms, batch_mean, batch_std, label):
    in_maps = shard_inputs(cosine, norms, batch_mean, batch_std, label)
    res = run_on_hw(in_maps)
    return unshard_output(res.results)


# revision 19
# speedup vs baseline: 1.5579x; 1.4088x over previous
"""AdaFace margin loss on 8 trn2 NeuronCores (class-dim sharded, partial-FC style).

Key identity: for non-label entries the reference computes
cos(arccos(c)) * S == c * S, so the bulk of the [512, 85742] output is a
pure scale -- memory-bound streaming. Only the <=512 label positions need
the margin path, computed as cos(arccos(c) + g) = c*cos(g) - sqrt(1-c^2)*sin(g).

Label cosines are fetched with an indirect-DMA gather (out-of-shard labels
read a dummy input row so every index is in bounds). The margin is applied
additively while each bulk tile is in SBUF:

    fixup = (iota == label_col[row]) * (corrected - c_label) * S   # off path
    out   = tile * S + fixup                                       # one fused op

so the streaming loop carries exactly one vector op between load and store
and the output is written once -- no scatter, no read-modify-write tail.

All per-row quantities live in [128, 4] layout (row rb*128+p at (p, rb));
cross-partition statistics use gpsimd partition_all_reduce.
"""

import math

import ml_dtypes
import numpy as np

IO_BF16 = True
NP_IO = ml_dtypes.bfloat16 if IO_BF16 else np.float32

B = 512          # batch
C = 85742        # classes (global)
NCORES = 8
CS = 10720       # per-core shard width; 8*CS = 85760 >= C (padded)
P = 128          # SBUF partitions
NB = B // P      # 4 row blocks of 128 rows
W = 2680         # bulk tile free width
NCT = CS // W    # col tiles per row block
BULK_BUFS = 6
FIX_BUFS = 3

M_CONST = 0.4
H_CONST = 0.333
S_CONST = 64.0
T_ALPHA = 0.01
EPS = 0.001

_NC_CACHE = {}


def build_nc(w=W, bulk_bufs=BULK_BUFS, fix_bufs=FIX_BUFS, io_bf16=None):
    if io_bf16 is None:
        io_bf16 = IO_BF16
    import concourse.bass as bass
    import concourse.bass_isa as bass_isa
    import concourse.mybir as mybir
    from concourse.bacc import Bacc
    from concourse.tile import TileContext

    f32 = mybir.dt.float32
    i32 = mybir.dt.int32
    fio = mybir.dt.bfloat16 if io_bf16 else mybir.dt.float32
    Alu = mybir.AluOpType
    Act = mybir.ActivationFunctionType
    X = mybir.AxisListType.X

    nct = CS // w
    assert nct * w == CS

    nc = Bacc("TRN2", target_bir_lowering=False)
    # cosine carries one dummy row: out-of-shard labels gather from it, so
    # no gather index is ever out of bounds.
    cos_d = nc.declare_dram_parameter("cosine", [B + 1, CS], fio, isOutput=False)
    nrm_d = nc.declare_dram_parameter("norms", [P, NB], f32, isOutput=False)
    bm_d = nc.declare_dram_parameter("batch_mean", [P, 1], f32, isOutput=False)
    bs_d = nc.declare_dram_parameter("batch_std", [P, 1], f32, isOutput=False)
    fidx_d = nc.declare_dram_parameter("fidx", [P, NB], i32, isOutput=False)
    # colf[p, rb*nct+ct] = label col of row rb*128+p minus ct*w, or -1 when
    # the label is not in this shard/tile
    colf_d = nc.declare_dram_parameter("colf", [P, NB * nct], f32, isOutput=False)
    out_d = nc.declare_dram_parameter("out", [B, CS], fio, isOutput=True)

    cos_flat = cos_d[:, :].rearrange("b (c u) -> (b c) u", u=1)

    with TileContext(nc) as tc:
        with (
            tc.tile_pool(name="small", bufs=1) as sp,
            tc.tile_pool(name="bulk", bufs=bulk_bufs) as bp,
            tc.tile_pool(name="fix", bufs=fix_bufs) as fp,
        ):
            # ---- tiny inputs ------------------------------------------------
            fx_t = sp.tile([P, NB], i32)
            nc.sync.dma_start(out=fx_t[:], in_=fidx_d[:, :])
            n_t = sp.tile([P, NB], f32)
            nc.sync.dma_start(out=n_t[:], in_=nrm_d[:, :])
            bm_t = sp.tile([P, 1], f32)
            nc.sync.dma_start(out=bm_t[:], in_=bm_d[:, :])
            bs_t = sp.tile([P, 1], f32)
            nc.sync.dma_start(out=bs_t[:], in_=bs_d[:, :])
            cf_t = sp.tile([P, NB * nct], f32)
            nc.sync.dma_start(out=cf_t[:], in_=colf_d[:, :])

            # ---- gather label cosines first (frees the gpsimd queue) --------
            clab_io = sp.tile([P, NB], fio)
            for i in range(NB):
                nc.gpsimd.indirect_dma_start(
                    out=clab_io[:, i : i + 1],
                    out_offset=None,
                    in_=cos_flat,
                    in_offset=bass.IndirectOffsetOnAxis(
                        ap=fx_t[:, i : i + 1], axis=0
                    ),
                )

            clab_t = sp.tile([P, NB], f32)
            nc.vector.tensor_copy(clab_t[:], clab_io[:])

            # ---- iota 0..w-1 on DVE (prefix scan of ones) -------------------
            ones_t = sp.tile([P, 1], f32)
            nc.vector.memset(ones_t[:], 1.0)
            iota_t = sp.tile([P, w], f32)
            nc.vector.tensor_tensor_scan(
                out=iota_t[:],
                data0=ones_t[:].to_broadcast([P, w]),
                data1=ones_t[:].to_broadcast([P, w]),
                initial=-1.0,
                op0=Alu.add,
                op1=Alu.bypass,
            )

            # ---- norm statistics (safe_norms mean / unbiased std) -----------
            nc.vector.tensor_scalar(
                out=n_t[:], in0=n_t[:], scalar1=0.001, scalar2=100.0,
                op0=Alu.max, op1=Alu.min,
            )
            psum_t = sp.tile([P, 1], f32)
            nc.vector.reduce_sum(out=psum_t[:], in_=n_t[:], axis=X)
            nc.gpsimd.partition_all_reduce(
                psum_t[:], psum_t[:], P, bass_isa.ReduceOp.add
            )
            mean_t = sp.tile([P, 1], f32)
            nc.vector.tensor_scalar_mul(mean_t[:], psum_t[:], 1.0 / B)
            cen_t = sp.tile([P, NB], f32)
            nc.vector.tensor_tensor(
                out=cen_t[:], in0=n_t[:], in1=mean_t[:].to_broadcast([P, NB]),
                op=Alu.subtract,
            )
            sq_t = sp.tile([P, NB], f32)
            nc.scalar.activation(sq_t[:], cen_t[:], Act.Square)
            vs_t = sp.tile([P, 1], f32)
            nc.vector.reduce_sum(out=vs_t[:], in_=sq_t[:], axis=X)
            nc.gpsimd.partition_all_reduce(
                vs_t[:], vs_t[:], P, bass_isa.ReduceOp.add
            )
            std_t = sp.tile([P, 1], f32)
            nc.scalar.activation(std_t[:], vs_t[:], Act.Sqrt, scale=1.0 / (B - 1))

            # ---- EMA buffers + margin scaler --------------------------------
            nm_t = sp.tile([P, 1], f32)
            nc.vector.tensor_scalar_mul(nm_t[:], mean_t[:], T_ALPHA)
            tb_t = sp.tile([P, 1], f32)
            nc.vector.tensor_scalar_mul(tb_t[:], bm_t[:], 1.0 - T_ALPHA)
            nc.vector.tensor_add(nm_t[:], nm_t[:], tb_t[:])
            dn_t = sp.tile([P, 1], f32)
            nc.vector.tensor_scalar(
                out=dn_t[:], in0=std_t[:], scalar1=T_ALPHA, scalar2=EPS,
                op0=Alu.mult, op1=Alu.add,
            )
            ts_t = sp.tile([P, 1], f32)
            nc.vector.tensor_scalar_mul(ts_t[:], bs_t[:], 1.0 - T_ALPHA)
            nc.vector.tensor_add(dn_t[:], dn_t[:], ts_t[:])
            inv_t = sp.tile([P, 1], f32)
            nc.vector.reciprocal(inv_t[:], dn_t[:])

            ms_t = sp.tile([P, NB], f32)
            nc.vector.tensor_tensor(
                out=ms_t[:], in0=n_t[:], in1=nm_t[:].to_broadcast([P, NB]),
                op=Alu.subtract,
            )
            nc.vector.tensor_tensor(
                out=ms_t[:], in0=ms_t[:], in1=inv_t[:].to_broadcast([P, NB]),
                op=Alu.mult,
            )
            nc.vector.tensor_scalar(
                out=ms_t[:], in0=ms_t[:], scalar1=H_CONST, scalar2=None,
                op0=Alu.mult,
            )
            nc.vector.tensor_scalar(
                out=ms_t[:], in0=ms_t[:], scalar1=-1.0, scalar2=1.0,
                op0=Alu.max, op1=Alu.min,
            )
            ga_t = sp.tile([P, NB], f32)
            nc.vector.tensor_scalar_mul(ga_t[:], ms_t[:], -M_CONST)
            cosg_t = sp.tile([P, NB], f32)
            nc.vector.tensor_scalar_add(cosg_t[:], ga_t[:], math.pi / 2.0)
            nc.scalar.activation(cosg_t[:], cosg_t[:], Act.Sin)
            sing_t = sp.tile([P, NB], f32)
            nc.scalar.activation(sing_t[:], ga_t[:], Act.Sin)
            gadd_t = sp.tile([P, NB], f32)
            nc.vector.tensor_scalar(
                out=gadd_t[:], in0=ms_t[:], scalar1=M_CONST, scalar2=M_CONST,
                op0=Alu.mult, op1=Alu.add,
            )

            # ---- additive correction: delta = S * (corrected - c_label) -----
            c2_t = sp.tile([P, NB], f32)
            nc.scalar.activation(c2_t[:], clab_t[:], Act.Square)
            sn_t = sp.tile([P, NB], f32)
            nc.scalar.activation(sn_t[:], c2_t[:], Act.Sqrt, bias=1.0, scale=-1.0)
            delta_t = sp.tile([P, NB], f32)
            nc.vector.tensor_mul(delta_t[:], clab_t[:], cosg_t[:])
            nc.vector.tensor_mul(sn_t[:], sn_t[:], sing_t[:])
            nc.vector.tensor_sub(delta_t[:], delta_t[:], sn_t[:])
            nc.vector.tensor_sub(delta_t[:], delta_t[:], gadd_t[:])
            nc.vector.tensor_sub(delta_t[:], delta_t[:], clab_t[:])
            nc.vector.tensor_scalar_mul(delta_t[:], delta_t[:], S_CONST)

            # ---- bulk: out = cosine * S + (iota==label_col)*delta ----------
            for rb in range(NB):
                rows = slice(rb * P, (rb + 1) * P)
                for ct in range(nct):
                    cols = slice(ct * w, (ct + 1) * w)
                    tidx = rb * nct + ct
                    fixup_t = fp.tile([P, w], f32)
                    nc.vector.tensor_scalar(
                        out=fixup_t[:], in0=iota_t[:],
                        scalar1=cf_t[:, tidx : tidx + 1],
                        scalar2=delta_t[:, rb : rb + 1],
                        op0=Alu.is_equal, op1=Alu.mult,
                    )
                    tile = bp.tile([P, w], fio)
                    nc.sync.dma_start(out=tile[:], in_=cos_d[rows, cols])
                    nc.vector.scalar_tensor_tensor(
                        out=tile[:], in0=tile[:], scalar=S_CONST,
                        in1=fixup_t[:], op0=Alu.mult, op1=Alu.add,
                    )
                    nc.scalar.dma_start(out=out_d[rows, cols], in_=tile[:])

    nc.finalize()  # Bacc.compile(): splits >1-wait instructions into EventSemaphores
    return nc


def get_nc():
    if "nc" not in _NC_CACHE:
        _NC_CACHE["nc"] = build_nc()
    return _NC_CACHE["nc"]


def shard_inputs(cosine, norms, batch_mean, batch_std, label):
    cosine = np.ascontiguousarray(np.asarray(cosine), dtype=np.float32)
    norms_pi = np.ascontiguousarray(
        np.asarray(norms, dtype=np.float32).reshape(NB, P).T
    )
    bm = np.full((P, 1), np.asarray(batch_mean, dtype=np.float32).reshape(-1)[0],
                 dtype=np.float32)
    bs = np.full((P, 1), np.asarray(batch_std, dtype=np.float32).reshape(-1)[0],
                 dtype=np.float32)
    lab = np.asarray(label).astype(np.int64).reshape(B)
    b_idx = np.arange(B, dtype=np.int64)
    nct = CS // W

    in_maps = []
    for k in range(NCORES):
        lo = k * CS
        hi = min(lo + CS, C)
        shard = np.zeros((B + 1, CS), dtype=NP_IO)
        shard[:B, : hi - lo] = cosine[:, lo:hi].astype(NP_IO)
        loc = lab - lo
        ok = (lab != -1) & (loc >= 0) & (loc < CS)
        # gather index; invalid rows read the dummy row (always in bounds)
        fidx = np.where(ok, b_idx * CS + loc, B * CS + b_idx).astype(np.int32)
        fidx_pi = np.ascontiguousarray(fidx.reshape(NB, P).T)
        # per-tile label column (or -1): row rb*128+p, tile tidx=rb*nct+ct
        colf = np.full((P, NB * nct), -1.0, dtype=np.float32)
        for rb in range(NB):
            rows = np.arange(rb * P, (rb + 1) * P)
            for ct in range(nct):
                cc = loc[rows] - ct * W
                m = ok[rows] & (cc >= 0) & (cc < W)
                colf[:, rb * nct + ct] = np.where(m, cc, -1).astype(np.float32)
        in_maps.append(
            {
                "cosine": shard,
                "norms": norms_pi,
                "batch_mean": bm,
                "batch_std": bs,
                "fidx": fidx_pi,
                "colf": np.ascontiguousarray(colf),
            }
        )
    return in_maps


def unshard_output(outs):
    full = np.empty((B, C), dtype=np.float32)
    for k in range(NCORES):
        lo = k * CS
        hi = min(lo + CS, C)
        full[:, lo:hi] = outs[k]["out"][:B, : hi - lo].astype(np.float32)
    return full


def run_on_hw(in_maps, trace=False, **kwargs):
    from concourse.bass_utils import run_bass_kernel_spmd

    nc = get_nc()
    return run_bass_kernel_spmd(
        nc, in_maps, core_ids=list(range(NCORES)), trace=trace, **kwargs
    )


def kernel(cosine, norms, batch_mean, batch_std, label):
    in_maps = shard_inputs(cosine, norms, batch_mean, batch_std, label)
    res = run_on_hw(in_maps)
    return unshard_output(res.results)


# revision 38
# speedup vs baseline: 1.6982x; 1.0901x over previous
"""AdaFace margin loss on 8 trn2 NeuronCores (class-dim sharded, partial-FC style).

Key identity: off the label column the reference computes
cos(arccos(c)) * S == c * S, so the bulk of the [512, 85742] output is a
pure scale -- memory-bound streaming (done in bf16: S = 64 = 2^6, so the
scale itself is exact; I/O rounding keeps every element within ~0.4%
relative, far inside the accuracy gate). Only the <=512 label positions
need the margin path, cos(arccos(c) + g) = c*cos(g) - sqrt(1-c^2)*sin(g).

Structure per core (shard = 10720 classes, padded from 85742/8):
  * all 16 bulk in-DMAs dispatch first on the Sync HWDGE ring (16 tile
    buffers, so the input stream free-runs at HBM rate); the tiny inputs
    go through the scalar-engine HWDGE ring so they are not starved.
  * DVE does exactly one 4x-mode tensor_scalar (x64) per tile; out-DMAs
    dispatch from the scalar engine.
  * norm statistics (mean / unbiased std / EMA / margin scaler) run in
    [128, 4] layout with gpsimd partition_all_reduce for cross-partition
    sums, overlapped with the stream.
  * label cosines come from four [128,1]-offset indirect-DMA gathers
    (out-of-shard labels read a dummy input row so no index is ever out
    of bounds); the corrected logits are scattered back with four [128,1]
    indirect DMAs, one per row block.
  * the output is split into four per-row-block DRAM tensors so each
    scatter's write-after-write dependency is exactly its own block's
    out-DMAs -- ordering is structural and the scatters pipeline with the
    bulk stream. Host-side unshard concatenates the blocks.

Hardware quirks baked in (sim accepts more than silicon does):
  * indirect DMA offsets/payloads must be one-per-partition [128,1] APs
    at a tile base (multi-column offset tables and 2-byte payload APs
    with a free offset are silently misread by the descriptor generator),
  * a DMA instruction carries at most one sync wait (Bacc's
    generate_event_semaphores legalizes the rest),
  * ACT tables and the gpsimd custom-op library are warmed up front.
"""

import math

import ml_dtypes
import numpy as np

IO_BF16 = True
NP_IO = ml_dtypes.bfloat16 if IO_BF16 else np.float32

B = 512          # batch
C = 85742        # classes (global)
NCORES = 8
CS = 10720       # per-core shard width; 8*CS = 85760 >= C (padded)
P = 128          # SBUF partitions
NB = B // P      # 4 row blocks of 128 rows
W = 2680         # bulk tile free width
NCT = CS // W    # col tiles per row block
BULK_BUFS = 16
FIX_BUFS = 4

M_CONST = 0.4
H_CONST = 0.333
S_CONST = 64.0
T_ALPHA = 0.01
EPS = 0.001

_NC_CACHE = {}


def build_nc(w=W, bulk_bufs=BULK_BUFS, fix_bufs=FIX_BUFS, io_bf16=None):
    if io_bf16 is None:
        io_bf16 = IO_BF16
    import concourse.bass as bass
    import concourse.bass_isa as bass_isa
    import concourse.mybir as mybir
    from concourse.bacc import Bacc
    from concourse.tile import TileContext

    f32 = mybir.dt.float32
    i32 = mybir.dt.int32
    fio = mybir.dt.bfloat16 if io_bf16 else mybir.dt.float32
    Alu = mybir.AluOpType
    Act = mybir.ActivationFunctionType
    X = mybir.AxisListType.X

    nct = CS // w
    assert nct * w == CS

    nc = Bacc("TRN2", target_bir_lowering=False)
    # cosine carries one dummy row: out-of-shard labels gather from it, so
    # no gather index is ever out of bounds.
    cos_d = nc.declare_dram_parameter("cosine", [B + 1, CS], fio, isOutput=False)
    nrm_d = nc.declare_dram_parameter("norms", [P, NB], f32, isOutput=False)
    bm_d = nc.declare_dram_parameter("batch_mean", [P, 1], f32, isOutput=False)
    bs_d = nc.declare_dram_parameter("batch_std", [P, 1], f32, isOutput=False)
    fidx_d = nc.declare_dram_parameter("fidx", [P, NB], i32, isOutput=False)
    sidx_d = nc.declare_dram_parameter("sidx", [P, NB], i32, isOutput=False)
    out_ds = [
        nc.declare_dram_parameter(f"out{rb}", [P + 1, CS], fio, isOutput=True)
        for rb in range(NB)
    ]

    cos_flat = cos_d[:, :].rearrange("b (c u) -> (b c) u", u=1)
    out_flats = [
        o[:, :].rearrange("b (c u) -> (b c) u", u=1) for o in out_ds
    ]

    with TileContext(nc) as tc:
        with (
            tc.tile_pool(name="small", bufs=1) as sp,
            tc.tile_pool(name="bulk", bufs=bulk_bufs) as bp,
        ):
            # ---- phase 1: dispatch every bulk in-DMA immediately ------------
            bulk_tiles = []
            for rb in range(NB):
                rows = slice(rb * P, (rb + 1) * P)
                for ct in range(nct):
                    cols = slice(ct * w, (ct + 1) * w)
                    tile = bp.tile([P, w], fio, tag="bulk")
                    nc.sync.dma_start(out=tile[:], in_=cos_d[rows, cols])
                    bulk_tiles.append((rb, ct, tile))

            # ---- tiny inputs (scalar-engine HWDGE: the Sync ring is full of
            # bulk ins and would starve these behind the whole stream) -------
            fx_t = sp.tile([P, NB], i32)
            nc.scalar.dma_start(out=fx_t[:], in_=fidx_d[:, :])
            sx_t = sp.tile([P, NB], i32)
            nc.scalar.dma_start(out=sx_t[:], in_=sidx_d[:, :])
            n_t = sp.tile([P, NB], f32)
            nc.scalar.dma_start(out=n_t[:], in_=nrm_d[:, :])
            bm_t = sp.tile([P, 1], f32)
            nc.scalar.dma_start(out=bm_t[:], in_=bm_d[:, :])
            bs_t = sp.tile([P, 1], f32)
            nc.scalar.dma_start(out=bs_t[:], in_=bs_d[:, :])

            # ---- warm-up: load ACT tables + gpsimd library during preamble --
            warm_t = sp.tile([P, 1], f32)
            nc.vector.memset(warm_t[:], 0.5)
            wo_t = sp.tile([P, 1], f32)
            nc.scalar.activation(wo_t[:], warm_t[:], Act.Sqrt)
            nc.scalar.activation(wo_t[:], warm_t[:], Act.Sin)
            nc.gpsimd.partition_all_reduce(
                wo_t[:], wo_t[:], P, bass_isa.ReduceOp.add
            )

            # ---- norm statistics (safe_norms mean / unbiased std) -----------
            nc.vector.tensor_scalar(
                out=n_t[:], in0=n_t[:], scalar1=0.001, scalar2=100.0,
                op0=Alu.max, op1=Alu.min,
            )
            psum_t = sp.tile([P, 1], f32)
            nc.vector.reduce_sum(out=psum_t[:], in_=n_t[:], axis=X)
            nc.gpsimd.partition_all_reduce(
                psum_t[:], psum_t[:], P, bass_isa.ReduceOp.add
            )
            mean_t = sp.tile([P, 1], f32)
            nc.vector.tensor_scalar_mul(mean_t[:], psum_t[:], 1.0 / B)
            cen_t = sp.tile([P, NB], f32)
            nc.vector.tensor_tensor(
                out=cen_t[:], in0=n_t[:], in1=mean_t[:].to_broadcast([P, NB]),
                op=Alu.subtract,
            )
            sq_t = sp.tile([P, NB], f32)
            nc.vector.tensor_mul(sq_t[:], cen_t[:], cen_t[:])
            vs_t = sp.tile([P, 1], f32)
            nc.vector.reduce_sum(out=vs_t[:], in_=sq_t[:], axis=X)
            nc.gpsimd.partition_all_reduce(
                vs_t[:], vs_t[:], P, bass_isa.ReduceOp.add
            )
            std_t = sp.tile([P, 1], f32)
            nc.scalar.activation(std_t[:], vs_t[:], Act.Sqrt, scale=1.0 / (B - 1))

            # ---- EMA buffers + margin scaler --------------------------------
            nm_t = sp.tile([P, 1], f32)
            nc.vector.tensor_scalar_mul(nm_t[:], mean_t[:], T_ALPHA)
            tb_t = sp.tile([P, 1], f32)
            nc.vector.tensor_scalar_mul(tb_t[:], bm_t[:], 1.0 - T_ALPHA)
            nc.vector.tensor_add(nm_t[:], nm_t[:], tb_t[:])
            dn_t = sp.tile([P, 1], f32)
            nc.vector.tensor_scalar(
                out=dn_t[:], in0=std_t[:], scalar1=T_ALPHA, scalar2=EPS,
                op0=Alu.mult, op1=Alu.add,
            )
            ts_t = sp.tile([P, 1], f32)
            nc.vector.tensor_scalar_mul(ts_t[:], bs_t[:], 1.0 - T_ALPHA)
            nc.vector.tensor_add(dn_t[:], dn_t[:], ts_t[:])
            inv_t = sp.tile([P, 1], f32)
            nc.vector.reciprocal(inv_t[:], dn_t[:])

            ms_t = sp.tile([P, NB], f32)
            nc.vector.tensor_tensor(
                out=ms_t[:], in0=n_t[:], in1=nm_t[:].to_broadcast([P, NB]),
                op=Alu.subtract,
            )
            nc.vector.tensor_tensor(
                out=ms_t[:], in0=ms_t[:], in1=inv_t[:].to_broadcast([P, NB]),
                op=Alu.mult,
            )
            nc.vector.tensor_scalar(
                out=ms_t[:], in0=ms_t[:], scalar1=H_CONST, scalar2=None,
                op0=Alu.mult,
            )
            nc.vector.tensor_scalar(
                out=ms_t[:], in0=ms_t[:], scalar1=-1.0, scalar2=1.0,
                op0=Alu.max, op1=Alu.min,
            )
            ga_t = sp.tile([P, NB], f32)
            nc.vector.tensor_scalar_mul(ga_t[:], ms_t[:], -M_CONST)
            # ---- gather label cosines (after the par-reduces in Pool order
            # so the stats chain is not stuck behind 20us of serial gathers) --
            clab_ios = []
            for i in range(NB):
                ci_t = sp.tile([P, 1], fio, tag=f"clab_io{i}")
                nc.gpsimd.indirect_dma_start(
                    out=ci_t[:],
                    out_offset=None,
                    in_=cos_flat,
                    in_offset=bass.IndirectOffsetOnAxis(
                        ap=fx_t[:, i : i + 1], axis=0
                    ),
                )
                clab_ios.append(ci_t)
            gadd_t = sp.tile([P, NB], f32)
            nc.vector.tensor_scalar(
                out=gadd_t[:], in0=ms_t[:], scalar1=M_CONST, scalar2=M_CONST,
                op0=Alu.mult, op1=Alu.add,
            )

            # ---- phase 2: out = cosine * S ---------------------------------
            for rb, ct, tile in bulk_tiles:
                cols = slice(ct * w, (ct + 1) * w)
                nc.vector.tensor_scalar_mul(tile[:], tile[:], S_CONST)
                nc.scalar.dma_start(out=out_ds[rb][:P, cols], in_=tile[:])

            # ---- corrected logits (gather-dependent; emitted after the bulk
            # loop so the DVE scale stream is never blocked behind it) -------
            clab_t = sp.tile([P, NB], f32)
            for i in range(NB):
                nc.vector.tensor_copy(clab_t[:, i : i + 1], clab_ios[i][:])
            c2_t = sp.tile([P, NB], f32)
            nc.vector.tensor_mul(c2_t[:], clab_t[:], clab_t[:])
            sn_t = sp.tile([P, NB], f32)
            nc.scalar.activation(sn_t[:], c2_t[:], Act.Sqrt, bias=1.0, scale=-1.0)
            cosg_t = sp.tile([P, NB], f32)
            nc.vector.tensor_scalar_add(cosg_t[:], ga_t[:], math.pi / 2.0)
            nc.scalar.activation(cosg_t[:], cosg_t[:], Act.Sin)
            sing_t = sp.tile([P, NB], f32)
            nc.scalar.activation(sing_t[:], ga_t[:], Act.Sin)
            delta_t = sp.tile([P, NB], f32)
            nc.vector.tensor_mul(delta_t[:], clab_t[:], cosg_t[:])
            nc.vector.tensor_mul(sn_t[:], sn_t[:], sing_t[:])
            nc.vector.tensor_sub(delta_t[:], delta_t[:], sn_t[:])
            nc.vector.tensor_sub(delta_t[:], delta_t[:], gadd_t[:])
            corr_io = sp.tile([P, NB], fio)
            nc.vector.tensor_scalar_mul(corr_io[:], delta_t[:], S_CONST)

            # ---- scatter the corrected logits onto the label positions -----
            # (out-of-shard rows land in each block tensor's dummy row).
            # Per-block output tensors make each scatter's DRAM-WAW exactly
            # its own block's out-DMAs: ordering is structural and the four
            # scatters pipeline with the bulk stream.
            # Stage each row block's payload at its own tile base: the HW
            # descgen drops the value-AP free offset for 2-byte dtypes.
            corr_cols = []
            for i in range(NB):
                cc_t = sp.tile([P, 1], fio, tag=f"corr_col{i}")
                nc.vector.tensor_copy(cc_t[:], corr_io[:, i : i + 1])
                corr_cols.append(cc_t)
            for i in range(NB):
                nc.gpsimd.indirect_dma_start(
                    out=out_flats[i],
                    out_offset=bass.IndirectOffsetOnAxis(
                        ap=sx_t[:, i : i + 1], axis=0
                    ),
                    in_=corr_cols[i][:],
                    in_offset=None,
                )

    nc.finalize()  # Bacc.compile(): splits >1-wait instructions into EventSemaphores
    return nc


def get_nc():
    if "nc" not in _NC_CACHE:
        _NC_CACHE["nc"] = build_nc()
    return _NC_CACHE["nc"]


def shard_inputs(cosine, norms, batch_mean, batch_std, label):
    cosine = np.ascontiguousarray(np.asarray(cosine), dtype=np.float32)
    norms_pi = np.ascontiguousarray(
        np.asarray(norms, dtype=np.float32).reshape(NB, P).T
    )
    bm = np.full((P, 1), np.asarray(batch_mean, dtype=np.float32).reshape(-1)[0],
                 dtype=np.float32)
    bs = np.full((P, 1), np.asarray(batch_std, dtype=np.float32).reshape(-1)[0],
                 dtype=np.float32)
    lab = np.asarray(label).astype(np.int64).reshape(B)
    b_idx = np.arange(B, dtype=np.int64)
    nct = CS // W

    in_maps = []
    for k in range(NCORES):
        lo = k * CS
        hi = min(lo + CS, C)
        shard = np.zeros((B + 1, CS), dtype=NP_IO)
        shard[:B, : hi - lo] = cosine[:, lo:hi].astype(NP_IO)
        loc = lab - lo
        ok = (lab != -1) & (loc >= 0) & (loc < CS)
        # gather index; invalid rows read the dummy row (always in bounds)
        gidx = np.where(ok, b_idx * CS + loc, B * CS + b_idx).astype(np.int32)
        gidx_pi = np.ascontiguousarray(gidx.reshape(NB, P).T)
        # scatter index within each block's own output tensor ([P+1, CS])
        p_idx = np.arange(P, dtype=np.int64)
        sidx = np.empty((P, NB), dtype=np.int32)
        for i in range(NB):
            loc_i = loc[i * P : (i + 1) * P]
            ok_i = ok[i * P : (i + 1) * P]
            sidx[:, i] = np.where(ok_i, p_idx * CS + loc_i, P * CS + p_idx)
        in_maps.append(
            {
                "cosine": shard,
                "norms": norms_pi,
                "batch_mean": bm,
                "batch_std": bs,
                "fidx": gidx_pi,
                "sidx": np.ascontiguousarray(sidx),
            }
        )
    return in_maps


def unshard_output(outs):
    full = np.empty((B, C), dtype=np.float32)
    for k in range(NCORES):
        lo = k * CS
        hi = min(lo + CS, C)
        for rb in range(NB):
            full[rb * P : (rb + 1) * P, lo:hi] = outs[k][f"out{rb}"][
                :P, : hi - lo
            ].astype(np.float32)
    return full


def run_on_hw(in_maps, trace=False, **kwargs):
    from concourse.bass_utils import run_bass_kernel_spmd

    nc = get_nc()
    return run_bass_kernel_spmd(
        nc, in_maps, core_ids=list(range(NCORES)), trace=trace, **kwargs
    )


def kernel(cosine, norms, batch_mean, batch_std, label):
    in_maps = shard_inputs(cosine, norms, batch_mean, batch_std, label)
    res = run_on_hw(in_maps)
    return unshard_output(res.results)
